# revision 2
# baseline (speedup 1.0000x reference)
"""AdditiveAttention kernel for 8 TRN2 NeuronCores (data-parallel over batch).

reference:
  q_proj = query @ Wq_w.T + Wq_b          [B, S, D]
  k_proj = value @ Wk_w.T + Wk_b          [B, S, D]
  scores = tanh(q_proj + k_proj) @ V_w[0] + V_b[0]     [B, S]
  attn   = softmax(scores, axis=-1)
  ctx    = attn[..., None] * value
  returns (ctx, attn)

Device layout (per core, 4 batches):
  - combined[e, tok] accumulated in PSUM from 8 matmuls (4 k-tiles x {Wq,Wk})
    with pre-transposed bf16 activations (d on partitions).
  - tanh + per-partition bias fused on ScalarE -> bf16.
  - scores[1, tok] = V_w-weighted partition sum via PE matmul (M=1).
  - softmax without max-subtraction (|scores| <= sum|V_w| + |V_b| ~ 23,
    exp is safe in f32; V_b cancels in softmax and is dropped).
  - attn row transposed to [128, 32] via PE transpose so attn becomes a
    per-partition scalar for the context multiply on VectorE.
"""

import os
import sys
import types

sys.path.insert(0, "/opt/trn_rl_repo")

import numpy as np
import ml_dtypes

B, S, D = 32, 4096, 512
NCORES = 8
B_LOC = B // NCORES          # 4 batches per core
T = B_LOC * S                # 16384 tokens per core
P = 128
KD = D // P                  # 4 contraction tiles
ET = D // P                  # 4 output-feature tiles
HALF = S // 4                # 1024-token activation load granularity
CHUNK = 512                  # matmul moving free dim / PSUM bank
NQ = 4                       # context-phase quarters per batch
QT = S // P // NQ            # 8 token-tiles per quarter
BF16 = ml_dtypes.bfloat16

LAST_EXEC_NS = None

_cache = {}


def _install_trace_shims():
    """Make trace=True work under axon in this container: the antenv here
    lacks axon_hooks, and upload_artifacts needs S3."""
    try:
        if "antenv.axon_hooks" not in sys.modules:
            from trn_agent_boot.trn_boot import _ntff_profile_via_ctypes

            hook = _ntff_profile_via_ctypes("/opt/axon/libaxon_pjrt.so")
            mod = types.ModuleType("antenv.axon_hooks")
            mod.get_axon_ntff_profile_hook = lambda: hook
            mod.set_axon_ntff_profile_hook = lambda h: None
            sys.modules["antenv.axon_hooks"] = mod
        import concourse.bass_utils as bu

        bu.upload_artifacts = lambda tmpdir: tmpdir
    except Exception:
        pass


def _build_nc():
    import concourse.tile as tile
    from concourse import bacc, mybir
    from concourse.masks import make_identity

    f32 = mybir.dt.float32
    bf16 = mybir.dt.bfloat16
    Act = mybir.ActivationFunctionType

    nc = bacc.Bacc(None, target_bir_lowering=False)

    qT = nc.declare_dram_parameter("qT", [D, T], bf16, isOutput=False)
    vT = nc.declare_dram_parameter("vT", [D, T], bf16, isOutput=False)
    v = nc.declare_dram_parameter("v", [T, D], f32, isOutput=False)
    wq = nc.declare_dram_parameter("wq", [D, D], bf16, isOutput=False)  # [d, e]
    wk = nc.declare_dram_parameter("wk", [D, D], bf16, isOutput=False)  # [d, e]
    bias = nc.declare_dram_parameter("bias", [P, ET], f32, isOutput=False)
    vw = nc.declare_dram_parameter("vw", [P, ET], bf16, isOutput=False)
    out_ctx = nc.declare_dram_parameter("out_ctx", [T, D], f32, isOutput=True)
    out_attn = nc.declare_dram_parameter("out_attn", [B_LOC, S], f32, isOutput=True)

    with tile.TileContext(nc) as tc:
        with (
            tc.tile_pool(name="consts", bufs=1) as consts,
            tc.tile_pool(name="acts", bufs=2) as acts,
            tc.tile_pool(name="vload", bufs=2) as vload,
            tc.tile_pool(name="ctxp", bufs=2) as ctxp,
            tc.tile_pool(name="tanhp", bufs=2) as tanhp,
            tc.tile_pool(name="rows", bufs=1) as rows,
            tc.tile_pool(name="small", bufs=2) as small,
            tc.tile_pool(name="dramp", bufs=2, space="DRAM") as dramp,
            tc.tile_pool(name="ps_qk", bufs=3, space="PSUM") as ps_qk,
            tc.tile_pool(name="ps_s", bufs=2, space="PSUM") as ps_s,
            tc.tile_pool(name="ps_t", bufs=2, space="PSUM") as ps_t,
        ):
            wq_sb = consts.tile([P, KD, D], bf16)
            nc.sync.dma_start(wq_sb[:], wq.rearrange("(kd p) e -> p kd e", p=P))
            wk_sb = consts.tile([P, KD, D], bf16)
            nc.sync.dma_start(wk_sb[:], wk.rearrange("(kd p) e -> p kd e", p=P))
            bias_sb = consts.tile([P, ET], f32)
            nc.sync.dma_start(bias_sb[:], bias[:])
            vw_sb = consts.tile([P, ET], bf16)
            nc.sync.dma_start(vw_sb[:], vw[:])
            ident = consts.tile([P, P], f32)
            make_identity(nc, ident[:])

            qT_r = qT.rearrange("(kd p) t -> p kd t", p=P)
            vT_r = vT.rearrange("(kd p) t -> p kd t", p=P)
            v_r = v.rearrange("(n p) d -> p n d", p=P)
            ctx_r = out_ctx.rearrange("(n p) d -> p n d", p=P)

            for b in range(B_LOC):
                scores_row = rows.tile([1, S], f32, tag="scores")

                for h in range(S // HALF):
                    t0 = b * S + h * HALF
                    q_sb = acts.tile([P, KD, HALF], bf16, tag="q")
                    nc.sync.dma_start(q_sb[:], qT_r[:, :, t0 : t0 + HALF])
                    vt_sb = acts.tile([P, KD, HALF], bf16, tag="vt")
                    nc.sync.dma_start(vt_sb[:], vT_r[:, :, t0 : t0 + HALF])

                    for j in range(HALF // CHUNK):
                        c0 = j * CHUNK
                        tanh_sb = tanhp.tile([P, ET, CHUNK], bf16, tag="tanh")
                        for e in range(ET):
                            pq = ps_qk.tile([P, CHUNK], f32, tag="qk")
                            for kd in range(KD):
                                nc.tensor.matmul(
                                    pq[:],
                                    lhsT=wq_sb[:, kd, e * P : (e + 1) * P],
                                    rhs=q_sb[:, kd, c0 : c0 + CHUNK],
                                    start=(kd == 0),
                                    stop=False,
                                )
                            for kd in range(KD):
                                nc.tensor.matmul(
                                    pq[:],
                                    lhsT=wk_sb[:, kd, e * P : (e + 1) * P],
                                    rhs=vt_sb[:, kd, c0 : c0 + CHUNK],
                                    start=False,
                                    stop=(kd == KD - 1),
                                )
                            nc.scalar.activation(
                                tanh_sb[:, e, :],
                                pq[:],
                                Act.Tanh,
                                bias=bias_sb[:, e : e + 1],
                            )
                        ps = ps_s.tile([1, CHUNK], f32, tag="s")
                        for e in range(ET):
                            nc.tensor.matmul(
                                ps[:],
                                lhsT=vw_sb[:, e : e + 1],
                                rhs=tanh_sb[:, e, :],
                                start=(e == 0),
                                stop=(e == ET - 1),
                            )
                        nc.vector.tensor_copy(
                            scores_row[:, h * HALF + c0 : h * HALF + c0 + CHUNK], ps[:]
                        )

                # softmax over the 4096 scores of batch b (no max needed:
                # |scores| <= sum|V_w| ~ 23, exp stays finite in f32)
                ssum = small.tile([1, 1], f32, tag="ssum")
                attn_row = rows.tile([1, S], f32, tag="attn")
                nc.scalar.activation(
                    attn_row[:], scores_row[:], Act.Exp, accum_out=ssum[:]
                )
                inv = small.tile([1, 1], f32, tag="inv")
                nc.vector.reciprocal(inv[:], ssum[:])
                nc.vector.tensor_scalar_mul(attn_row[:], attn_row[:], inv[:])
                nc.sync.dma_start(out_attn[b : b + 1, :], attn_row[:])

                # [1, 4096] -> [32, 128] (DRAM bounce) -> PE transpose -> [128, 32]
                attn_d = dramp.tile([1, S], f32, tag="attn_d")
                nc.sync.dma_start(attn_d[:], attn_row[:])
                attn32 = small.tile([32, P], f32, tag="attn32")
                nc.sync.dma_start(
                    attn32[:], attn_d[0, :].rearrange("(c p) -> c p", p=P)
                )
                pt = ps_t.tile([P, 32], f32, tag="pt")
                nc.tensor.transpose(pt[:], attn32[:], ident[:32, :32])
                attn_t = small.tile([P, 32], f32, tag="attn_t")
                nc.vector.tensor_copy(attn_t[:], pt[:])

                for q in range(NQ):
                    n0 = b * (S // P) + q * QT
                    v_sb = vload.tile([P, QT, D], f32, tag="v")
                    nc.sync.dma_start(v_sb[:], v_r[:, n0 : n0 + QT, :])
                    ctx_sb = ctxp.tile([P, QT, D], f32, tag="ctx")
                    for n in range(QT):
                        col = q * QT + n
                        nc.vector.tensor_scalar_mul(
                            ctx_sb[:, n, :], v_sb[:, n, :], attn_t[:, col : col + 1]
                        )
                    nc.sync.dma_start(ctx_r[:, n0 : n0 + QT, :], ctx_sb[:])

    nc.finalize()
    return nc


def _get_nc():
    if "nc" not in _cache:
        _cache["nc"] = _build_nc()
    return _cache["nc"]


def kernel(query, value, Wq_w, Wq_b, Wk_w, Wk_b, V_w, V_b):
    global LAST_EXEC_NS
    _install_trace_shims()
    from concourse.bass_utils import run_bass_kernel_spmd

    query = np.asarray(query, dtype=np.float32)
    value = np.asarray(value, dtype=np.float32)
    wq_t = np.ascontiguousarray(np.asarray(Wq_w, np.float32).T).astype(BF16)
    wk_t = np.ascontiguousarray(np.asarray(Wk_w, np.float32).T).astype(BF16)
    bias_sum = (np.asarray(Wq_b, np.float32) + np.asarray(Wk_b, np.float32))
    bias_pack = np.ascontiguousarray(bias_sum.reshape(ET, P).T)  # [P, ET]
    vw_pack = np.ascontiguousarray(
        np.asarray(V_w, np.float32)[0].reshape(ET, P).T
    ).astype(BF16)  # [P, ET]

    in_maps = []
    for c in range(NCORES):
        qs = query[c * B_LOC : (c + 1) * B_LOC]  # [B_LOC, S, D]
        vs = value[c * B_LOC : (c + 1) * B_LOC]
        qT = np.ascontiguousarray(qs.transpose(2, 0, 1).reshape(D, T)).astype(BF16)
        vT = np.ascontiguousarray(vs.transpose(2, 0, 1).reshape(D, T)).astype(BF16)
        in_maps.append(
            {
                "qT": qT,
                "vT": vT,
                "v": np.ascontiguousarray(vs.reshape(T, D)),
                "wq": wq_t,
                "wk": wk_t,
                "bias": bias_pack,
                "vw": vw_pack,
            }
        )

    nc = _get_nc()
    trace = os.environ.get("KERNEL_TRACE") == "1"
    res = run_bass_kernel_spmd(nc, in_maps, core_ids=list(range(NCORES)), trace=trace)
    LAST_EXEC_NS = res.exec_time_ns

    ctx = np.concatenate(
        [res.results[c]["out_ctx"].reshape(B_LOC, S, D) for c in range(NCORES)], axis=0
    )
    attn = np.concatenate(
        [res.results[c]["out_attn"] for c in range(NCORES)], axis=0
    )
    return ctx, attn
